# revision 1
# baseline (speedup 1.0000x reference)
"""Trainium2 Bass kernel for nn_CRF_16389595202091.

CRF layer: dense projection [B,T,D]x[D,U] -> potentials, then Viterbi decode,
returning (pot, onehot(tags)).  Data-parallel over batch: 8 NeuronCores x 8
batch rows each.  All shapes hardcoded per the spec: B=64, T=1024, D=1024,
U=64, mask all-ones.

Per-core layout (b = local batch 0..7, bh=b//4, bl=b%4):
  potT      [64(u), 8192(n=b*1024+t)]      projection output
  potFlat3  [128(bh,t%64), (t//64, bl, u)] pot rows for the PE row-selector mms
  forward step s=1..1024 (s=1024 = virtual last-tag step, no trans term):
    ps_sc[p=(bh,j), (bl,i)] = state_{s-1}[(bh,bl), i] + trans[i, j]
      built by PE: mm-trans (I64x2 selector) + mm-m (per-bh row broadcast)
    maxfix[p,(bl)] = max_i ps_sc   (m_s);  bphist[:, s-1, :] = 63 - max_i
      (eq(ps_sc,maxfix) * (63-i))  = argmax_i with lowest-index tiebreak
    psum_row rows {0,64}: flatten mms (m_s) + e-selector pot-row mms = state_s
    st2t rows {0,64} <- psum_row (ACT copy)
  backtrace: bphist -> (PE transposes) -> bpnat16b [128(bh,bl,s16),(kk,x)] bf16
    -> (DMA) -> bpnat8 [8(b), (s16,kk,x)];  then 1024-step backward chase on
    partitions 0..7: eq = onehot(tag) (doubles as the onehot output column),
    scalar_tensor_tensor accum-sum gathers table[tag].
"""

import os
import numpy as np

STAGE = os.environ.get("CRF_STAGE", "E")  # A < C < D < E
FWDSTEPS = int(os.environ.get("CRF_FWDSTEPS", "1024"))
B, T, D, U = 64, 1024, 1024, 64
NB = B // 8          # batches per core
NTOK = NB * T        # tokens per core
TBLK = T // 64

_cached = {}


def _build_nc():
    import concourse.bass as bass
    import concourse.bacc as bacc
    import concourse.mybir as mybir
    from concourse.tile import TileContext

    f32 = mybir.dt.float32
    u8 = mybir.dt.uint8
    AX = mybir.AxisListType.X
    OP = mybir.AluOpType

    nc = bacc.Bacc("TRN2", target_bir_lowering=False, debug=False, num_devices=8)

    x_d = nc.dram_tensor("x", [NTOK, D], f32, kind="ExternalInput")
    w_d = nc.dram_tensor("w", [D, U], f32, kind="ExternalInput")
    bcol_d = nc.dram_tensor("bcol", [U, 1], f32, kind="ExternalInput")
    transT_d = nc.dram_tensor("transT", [U, U], f32, kind="ExternalInput")
    lr_d = nc.dram_tensor("lr", [U, 2], f32, kind="ExternalInput")
    i128_d = nc.dram_tensor("i128", [128, 128], f32, kind="ExternalInput")
    i64x2_d = nc.dram_tensor("i64x2", [64, 128], f32, kind="ExternalInput")
    selc_d = nc.dram_tensor("selc", [128, 128], f32, kind="ExternalInput")
    iota8_d = nc.dram_tensor("iota8", [8, 64], f32, kind="ExternalInput")
    iotarev_d = nc.dram_tensor("iotarev", [128, 64], f32, kind="ExternalInput")
    selhi_d = nc.dram_tensor("selhi", [64, 128], f32, kind="ExternalInput")

    pot_d = nc.dram_tensor("pot_out", [NB, T, U], f32, kind="ExternalOutput")
    oh_d = nc.dram_tensor("oh_out", [NB, T, U], f32, kind="ExternalOutput")

    with TileContext(nc) as tc:
        with tc.tile_pool(name="const", bufs=1) as cpool:
            # ---- constants ----
            i128 = cpool.tile([128, 128], f32)
            nc.sync.dma_start(i128[:], i128_d[:])
            i64dup = cpool.tile([128, 64], f32)
            nc.sync.dma_start(i64dup[0:64, :], i128_d[0:64, 0:64])
            nc.sync.dma_start(i64dup[64:128, :], i128_d[0:64, 0:64])
            i64x2 = cpool.tile([64, 128], f32)
            nc.sync.dma_start(i64x2[:], i64x2_d[:])
            selc = cpool.tile([128, 128], f32)
            nc.sync.dma_start(selc[:], selc_d[:])
            selhi = cpool.tile([64, 128], f32)
            nc.sync.dma_start(selhi[:], selhi_d[:])
            iota8 = cpool.tile([8, 64], f32)
            nc.sync.dma_start(iota8[:], iota8_d[:])
            iotarev = cpool.tile([128, 64], f32)
            nc.sync.dma_start(iotarev[:], iotarev_d[:])
            transT4 = cpool.tile([64, 256], f32)
            for _bl in range(4):
                nc.sync.dma_start(transT4[:, _bl * 64:(_bl + 1) * 64],
                                  transT_d[:])
            bcol = cpool.tile([64, 1], f32)
            nc.sync.dma_start(bcol[:], bcol_d[:])
            lrsb = cpool.tile([64, 2], f32)
            nc.sync.dma_start(lrsb[:], lr_d[:])
            wsb = cpool.tile([128, 8, 64], f32)
            nc.sync.dma_start(wsb[:], w_d[:].rearrange("(c p) u -> p c u", p=128))

            midp = tc.alloc_tile_pool(name="mid", bufs=1)
            potF3 = midp.tile([128, TBLK, 4, 64], f32)
            bphist = midp.tile([128, T, 4], f32)

            # ================= Stage A: projection =================
            potTp = tc.alloc_tile_pool(name="potTp", bufs=1)
            potT = potTp.tile([64, NTOK], f32)
            with tc.tile_pool(name="xin", bufs=3) as xinp, \
                 tc.tile_pool(name="xT", bufs=4) as xTp, \
                 tc.tile_pool(name="pstr", bufs=4, space="PSUM") as pstrp, \
                 tc.tile_pool(name="pspot", bufs=2, space="PSUM") as pspotp:
                for tt in range(NTOK // 128):
                    xtile = xinp.tile([128, D], f32)
                    nc.sync.dma_start(xtile[:], x_d[tt * 128:(tt + 1) * 128, :])
                    pspot = pspotp.tile([64, 128], f32)
                    for dc in range(8):
                        pstr = pstrp.tile([128, 128], f32)
                        nc.tensor.transpose(
                            pstr[:], xtile[:, dc * 128:(dc + 1) * 128], i128[:])
                        xT = xTp.tile([128, 128], f32)
                        if dc % 2 == 0:
                            nc.vector.tensor_copy(xT[:], pstr[:])
                        else:
                            nc.scalar.copy(xT[:], pstr[:])
                        nc.tensor.matmul(pspot[:], lhsT=wsb[:, dc, :], rhs=xT[:],
                                         start=(dc == 0), stop=(dc == 7))
                    # bias add + PSUM->SBUF
                    nc.vector.tensor_scalar(
                        potT[:, tt * 128:(tt + 1) * 128], pspot[:],
                        bcol[:], None, OP.add)

            # boundary energies at t=0 / t=T-1 (mask assumed all ones)
            if STAGE >= "2":
                first = potT[:].rearrange("u (b t) -> u b t", b=NB)[:, :, 0]
                nc.vector.tensor_scalar(first, first, lrsb[:, 0:1], None, OP.add)
                last = potT[:].rearrange("u (b t) -> u b t", b=NB)[:, :, T - 1]
                nc.vector.tensor_scalar(last, last, lrsb[:, 1:2], None, OP.add)

            # ================= Stage B: potFlat3 =================
            with tc.tile_pool(name="psf", bufs=2, space="PSUM") as psfp:
                for tblk in range(TBLK if STAGE >= '3' else 0):
                    psf = psfp.tile([128, 256], f32)
                    for bh in range(2):
                        for bl in range(4):
                            b = bh * 4 + bl
                            src = potT[:, b * T + tblk * 64: b * T + tblk * 64 + 64]
                            dst = psf[bh * 64:(bh + 1) * 64,
                                      bl * 64:(bl + 1) * 64]
                            if bh == 0:
                                nc.tensor.transpose(dst, src, i64dup[0:64, :])
                            else:
                                nc.tensor.matmul(dst, lhsT=src,
                                                 rhs=i64dup[0:64, :],
                                                 start=True, stop=True,
                                                 skip_group_check=True)
                    if tblk % 2 == 0:
                        nc.vector.tensor_copy(potF3[:, tblk, :, :], psf[:])
                    else:
                        nc.scalar.copy(potF3[:, tblk, :, :], psf[:])

            # potT no longer needed; emit pot output from potF3 now
            potTp.release()
            potdv = pot_d[:].rearrange("(bh bl) (tblk t) u -> bh bl t tblk u",
                                       bh=2, tblk=TBLK)
            for bh in range(2 if STAGE >= "4" else 0):
                for bl in range(4):
                    nc.sync.dma_start(potdv[bh, bl],
                                      potF3[bh * 64:(bh + 1) * 64, :, bl, :])

            # ================= Stage C: forward =================
            run_c = STAGE >= "C"
            with tc.tile_pool(name="st2", bufs=2) as st2p, \
                 tc.tile_pool(name="mx", bufs=3) as mxp, \
                 tc.tile_pool(name="eqb", bufs=3) as eqp, \
                 tc.tile_pool(name="pssc", bufs=2, space="PSUM") as psscp, \
                 tc.tile_pool(name="psrow", bufs=2, space="PSUM") as psrowp:

                def potrow_mms(psA, psB, s, start):
                    # per-bh psum rows (both base 0) += pot[b, s, :] (bl,u)
                    r, tblk = s % 64, s // 64
                    for bh, ps in ((0, psA), (1, psB)):
                        nc.tensor.matmul(
                            ps[0:1, :],
                            lhsT=i64dup[bh * 64:(bh + 1) * 64, r:r + 1],
                            rhs=potF3[bh * 64:(bh + 1) * 64, tblk, :, :],
                            start=start, stop=True, skip_group_check=True)

                # manual double-buffered per-bh state rows at base 0
                # (rows 1-63 must be 0.0 for the k=64 selector matmuls)
                st2bufs = []
                for _i in range(4):
                    st2x = st2p.tile([64, 256], f32, name=f"st2_{_i}",
                                     tag=f"st2_{_i}")
                    nc.vector.memset(st2x[:], 0.0)
                    st2bufs.append(st2x)
                st2lo = st2hi = None
                if STAGE >= "5":
                    psrowA = psrowp.tile([64, 256], f32, name="psrowA",
                                         tag="psrowA")
                    psrowB = psrowp.tile([64, 256], f32, name="psrowB",
                                         tag="psrowB")
                    potrow_mms(psrowA, psrowB, 0, True)
                    st2lo, st2hi = st2bufs[0], st2bufs[1]
                    nc.scalar.copy(st2lo[0:1, :], psrowA[0:1, :])
                    nc.scalar.copy(st2hi[0:1, :], psrowB[0:1, :])

                for s in range(1, min(FWDSTEPS, T) + 1 if STAGE >= 'C' else 1):
                    ps_sc = psscp.tile([128, 256], f32)
                    if s < T:
                        nc.tensor.matmul(
                            ps_sc[:], lhsT=i64x2[:], rhs=transT4[:],
                            start=True, stop=False)
                    nc.tensor.matmul(ps_sc[:], lhsT=selc[0:64, :],
                                     rhs=st2lo[:], start=(s == T), stop=False)
                    nc.tensor.matmul(ps_sc[:], lhsT=selhi[:],
                                     rhs=st2hi[:], start=False, stop=True)
                    maxfix = mxp.tile([128, 4], f32)
                    nc.vector.tensor_reduce(
                        maxfix[:], ps_sc[:].rearrange("p (a b) -> p a b", a=4),
                        axis=AX, op=OP.max)
                    # bp extraction (reversed-index encoding, fixed in bulk later)
                    eqb = eqp.tile([128, 256], f32)
                    nc.vector.tensor_tensor(
                        out=eqb[:], in0=ps_sc[:],
                        in1=maxfix[:].unsqueeze(2).broadcast_to([128, 4, 64]),
                        op=OP.is_ge)
                    nc.vector.tensor_tensor(
                        out=eqb[:], in0=eqb[:],
                        in1=iotarev[:].unsqueeze(1).broadcast_to([128, 4, 64]),
                        op=OP.mult)
                    nc.vector.tensor_reduce(
                        bphist[:, s - 1, :],
                        eqb[:].rearrange("p (a b) -> p a b", a=4),
                        axis=AX, op=OP.max)
                    if s < T:
                        psrowA = psrowp.tile([64, 256], f32, name="psrowA",
                                             tag="psrowA")
                        psrowB = psrowp.tile([64, 256], f32, name="psrowB",
                                             tag="psrowB")
                        for bh, ps in ((0, psrowA), (1, psrowB)):
                            for bl in range(4):
                                nc.tensor.matmul(
                                    ps[0:1, bl * 64:(bl + 1) * 64],
                                    lhsT=maxfix[bh * 64:(bh + 1) * 64, bl:bl + 1],
                                    rhs=i64dup[bh * 64:(bh + 1) * 64, :],
                                    start=(bl == 0), stop=False,
                                    skip_group_check=True)
                        potrow_mms(psrowA, psrowB, s, False)
                        st2lo = st2bufs[(s % 2) * 2]
                        st2hi = st2bufs[(s % 2) * 2 + 1]
                        nc.scalar.copy(st2lo[0:1, :], psrowA[0:1, :])
                        nc.scalar.copy(st2hi[0:1, :], psrowB[0:1, :])

            # ================= Stage D: bp fix + relayout =================
            dpool = tc.alloc_tile_pool(name="dpool", bufs=1, side="right")
            # bphist holds (63 - bp); fix in place
            bpview = bphist[:].rearrange("p t a -> p (t a)")
            if STAGE >= "D":
                nc.vector.tensor_scalar(bpview, bpview, -1.0, 63.0,
                                        OP.mult, OP.add)

            bpT1 = dpool.tile([64, 8, 16, 64], u8)
            with tc.tile_pool(name="psd", bufs=2, space="PSUM") as psdp:
                for w in range(16 if STAGE >= 'D' else 0):
                    psd = psdp.tile([64, 512], f32)
                    for bh in range(2):
                        for bl in range(4):
                            b = bh * 4 + bl
                            src = bphist[bh * 64:(bh + 1) * 64,
                                         w * 64:(w + 1) * 64, bl]
                            dst = psd[:, b * 64:(b + 1) * 64]
                            if bh == 0:
                                nc.tensor.transpose(dst, src, i64dup[0:64, :])
                            else:
                                nc.tensor.matmul(dst, lhsT=src,
                                                 rhs=i64dup[64:128, :],
                                                 start=True, stop=True,
                                                 skip_group_check=True)
                    if w % 2 == 0:
                        nc.vector.tensor_copy(bpT1[:, :, w, :],
                                              psd[:].rearrange(
                                                  "r (b x) -> r b x", b=8))
                    else:
                        nc.scalar.copy(bpT1[:, :, w, :],
                                       psd[:].rearrange(
                                           "r (b x) -> r b x", b=8))

            bpnat8 = dpool.tile([8, 64, 16, 64], u8)
            for b in range(8 if STAGE >= 'D' else 0):
                nc.sync.dma_start(bpnat8[b:b + 1, :, :, :],
                                  bpT1[:, b, :, :])

            # ================= Stage E: backward chase =================
            midp.release()
            with tc.tile_pool(name="oh", bufs=2) as ohp, \
                 tc.tile_pool(name="chs", bufs=1) as chp:
                tag8 = chp.tile([8, 1], f32)
                nc.vector.memset(tag8[:], 0.0)
                junk8 = chp.tile([8, 64], f32)
                eqjunk = chp.tile([8, 64], f32)
                CH = 64  # t-columns per onehot staging chunk
                ohtiles = {}
                for t in range(T - 1 if STAGE >= 'E' else -1, -1, -1):
                    # eq = onehot(tag_{t+1}) -> ohstage column t+1 (skip t=1023)
                    if t < T - 1:
                        ci, co = (t + 1) // CH, (t + 1) % CH
                        if ci not in ohtiles:
                            ohtiles[ci] = ohp.tile([8, CH, 64], f32, name="ohstage", tag="ohstage")
                        eqout = ohtiles[ci][:, co, :]
                    else:
                        eqout = eqjunk[:]
                    nc.vector.tensor_scalar(eqout, iota8[:], tag8[:], None,
                                            OP.is_equal)
                    # gather: tag8 <- sum(eq * bp_t[b, :])
                    nc.vector.scalar_tensor_tensor(
                        out=junk8[:], in0=eqout, scalar=1.0,
                        in1=bpnat8[:, t % 64, t // 64, :],
                        op0=OP.mult, op1=OP.mult, accum_out=tag8[:])
                    if t < T - 1 and co == 0 and ci > 0:
                        nc.sync.dma_start(
                            oh_d[:, ci * CH:(ci + 1) * CH, :],
                            ohtiles.pop(ci)[:])
                # final column t=0 completes chunk 0
                if STAGE >= "E":
                    nc.vector.tensor_scalar(ohtiles[0][:, 0, :], iota8[:],
                                            tag8[:], None, OP.is_equal)
                    nc.sync.dma_start(oh_d[:, 0:CH, :], ohtiles.pop(0)[:])
            dpool.release()


    nc.finalize()
    return nc


def _consts():
    i128 = np.eye(128, dtype=np.float32)
    i64x2 = np.tile(np.eye(64, dtype=np.float32), (1, 2))
    selc = np.zeros((128, 128), np.float32)
    selc[0, 0:64] = 1.0
    selc[64, 64:128] = 1.0
    selhi = np.zeros((64, 128), np.float32)
    selhi[0, 64:128] = 1.0
    iota8 = np.tile(np.arange(64, dtype=np.float32)[None, :], (8, 1))
    iotarev = np.tile((63 - np.arange(64, dtype=np.float32))[None, :], (128, 1))
    return i128, i64x2, selc, selhi, iota8, iotarev


def kernel(inputs, mask, W, b, trans, left_b, right_b):
    from concourse.bass_utils import run_bass_kernel_spmd

    if "nc" not in _cached:
        _cached["nc"] = _build_nc()
    nc = _cached["nc"]

    inputs = np.ascontiguousarray(np.asarray(inputs, np.float32))
    W = np.ascontiguousarray(np.asarray(W, np.float32))
    bvec = np.asarray(b, np.float32).reshape(U, 1)
    trans = np.asarray(trans, np.float32)
    lr = np.stack([np.asarray(left_b, np.float32),
                   np.asarray(right_b, np.float32)], axis=1)
    i128, i64x2, selc, selhi, iota8, iotarev = _consts()

    in_maps = []
    for c in range(8):
        shard = inputs[c * NB:(c + 1) * NB].reshape(NTOK, D)
        in_maps.append({
            "x": np.ascontiguousarray(shard),
            "w": W, "bcol": bvec,
            "transT": np.ascontiguousarray(trans.T),
            "lr": np.ascontiguousarray(lr),
            "i128": i128, "i64x2": i64x2, "selc": selc, "selhi": selhi,
            "iota8": np.ascontiguousarray(iota8),
            "iotarev": np.ascontiguousarray(iotarev),
        })

    trace = bool(int(os.environ.get("CRF_TRACE", "0")))
    res = run_bass_kernel_spmd(nc, in_maps, core_ids=list(range(8)),
                               trace=trace)
    if trace:
        print("HW exec time:", res.exec_time_ns, "ns")
        print("mean exec:", res.mean_exec_time_ns, "trace:",
              res.instructions_and_trace[1] if res.instructions_and_trace else None)
    pot = np.concatenate([r["pot_out"] for r in res.results], axis=0)
    onehot = np.concatenate([r["oh_out"] for r in res.results], axis=0)
    return pot, onehot



# revision 11
# speedup vs baseline: 2.1496x; 2.1496x over previous
"""Trainium2 Bass kernel for nn_CRF_16389595202091 (v2 design).

CRF: dense projection [B,T,D]x[D,U] -> potentials, Viterbi decode, return
(pot, onehot(tags)).  Data-parallel over batch: 8 cores x 8 batches.
Shapes hardcoded: B=64, T=1024, D=1024, U=64, mask all-ones.

v2 forward loop (per core, per step) -- no per-step broadcast matmuls:
  state lives as mstate [128, 4]: partition p=(g,i) (g=p//64, i=p%64),
  col h in 0..3; batch = M_par[g][h] with parity-alternating mapping
  (an involution pi: M1(g',h') = 4*(h'%2) + 2*(h'//2) + g').
  1. DVE TT: scores[p=(g,i), (h,j)] = mstate_bc + transbc (const)  [128,256]
  2. PE: 2x transpose halves -> ps_tr [p=(hh,j), (H,g,i)] PSUM [128,256]
  3. DVE reduce max over i -> maxfix' [128, 4] (cols c'=(H,g))
  4. DVE TT tiny: mstate' = maxfix' + potT2 col (parity slot view)
  5. bp (deferred 1 step): mfneg = -HUGE*maxfix (DVE ts);
     4x ACT: z1_c = Abs(HUGE*ps_tr_c + mfneg_c)  (exact: 0 at argmax)
     4x DVE ttr: bphist[:,s,c] = max_i(wrev - z1_c)  = 63 - argmax_i
     (tie-safe: exact-0 only at fp32-max entries; lowest-i wins like jnp.argmax)
  Final pseudo-step s=T with zero trans term yields argmax of the last state.

Then: bulk bp fix (63-x), parity-split PE transposes -> bpnat8 [8,...] u8,
1024-step backward chase (1 DVE stt per step, tags accumulated into taghist),
bulk onehot via is_equal TT on 128 partitions, DMA out.
"""

import os
import numpy as np

B, T, D, U = 64, 1024, 1024, 64
NB = B // 8          # batches per core
NTOK = NB * T        # tokens per core
FWDSTEPS = int(os.environ.get("CRF_FWDSTEPS", str(T)))
BPMODE = os.environ.get("CRF_BPMODE", "red")  # "ttr" | "red"
HUGE = float(2.0 ** 30)

_cached = {}

# batch mapping tables: M_par[g][h] = batch at (partition-half g, col h)
M0 = [[4 * g + h for h in range(4)] for g in range(2)]
M1 = [[4 * (h % 2) + 2 * (h // 2) + g for h in range(4)] for g in range(2)]
MS = [M0, M1]
# potT2 slot layout [128, 8, 1024]: half g slots 0..3 = M0[g], 4..7 = M1[g]
SLOTB = [M0[0] + M1[0], M0[1] + M1[1]]


def _build_nc():
    import concourse.bass as bass
    import concourse.bacc as bacc
    import concourse.mybir as mybir
    from concourse.tile import TileContext

    f32 = mybir.dt.float32
    u8 = mybir.dt.uint8
    AX = mybir.AxisListType.X
    OP = mybir.AluOpType
    AF = mybir.ActivationFunctionType

    nc = bacc.Bacc("TRN2", target_bir_lowering=False, debug=False, num_devices=8)

    x_d = nc.dram_tensor("x", [NTOK, D], f32, kind="ExternalInput")
    w_d = nc.dram_tensor("w", [D, U], f32, kind="ExternalInput")
    bcol_d = nc.dram_tensor("bcol", [U, 1], f32, kind="ExternalInput")
    lr_d = nc.dram_tensor("lr", [U, 2], f32, kind="ExternalInput")
    i128_d = nc.dram_tensor("i128", [128, 128], f32, kind="ExternalInput")
    transbc_d = nc.dram_tensor("transbc", [128, 256], f32, kind="ExternalInput")
    wrev_d = nc.dram_tensor("wrev", [128, 256], f32, kind="ExternalInput")
    iota8_d = nc.dram_tensor("iota8", [8, 64], f32, kind="ExternalInput")
    iotaU_d = nc.dram_tensor("iotaU", [128, 64], f32, kind="ExternalInput")

    pot_d = nc.dram_tensor("pot_out", [NB, T, U], f32, kind="ExternalOutput")
    oh_d = nc.dram_tensor("oh_out", [NB, T, U], f32, kind="ExternalOutput")

    with TileContext(nc) as tc:
        with tc.tile_pool(name="const", bufs=1) as cpool:
            i128 = cpool.tile([128, 128], f32)
            nc.sync.dma_start(i128[:], i128_d[:])
            i64dup = cpool.tile([128, 64], f32)
            nc.sync.dma_start(i64dup[0:64, :], i128_d[0:64, 0:64])
            nc.sync.dma_start(i64dup[64:128, :], i128_d[0:64, 0:64])
            transbc = cpool.tile([128, 4, 64], f32)
            nc.sync.dma_start(transbc[:], transbc_d[:].rearrange(
                "p (a b) -> p a b", a=4))
            wrev = cpool.tile([128, 4, 64], f32)
            nc.sync.dma_start(wrev[:], wrev_d[:].rearrange(
                "p (a b) -> p a b", a=4))
            iota8 = cpool.tile([8, 64], f32)
            nc.sync.dma_start(iota8[:], iota8_d[:])
            iotaU = cpool.tile([128, 64], f32)
            nc.sync.dma_start(iotaU[:], iotaU_d[:])
            bcol = cpool.tile([64, 1], f32)
            nc.sync.dma_start(bcol[:], bcol_d[:])
            lrsb = cpool.tile([64, 2], f32)
            nc.sync.dma_start(lrsb[:], lr_d[:])
            wsb = cpool.tile([128, 8, 64], f32)
            nc.sync.dma_start(wsb[:], w_d[:].rearrange("(c p) u -> p c u", p=128))
            zero4 = cpool.tile([128, 4, 64], f32)
            nc.vector.memset(zero4[:], 0.0)

            midp = tc.alloc_tile_pool(name="mid", bufs=1)
            potT2 = midp.tile([128, 8, 1024], f32)
            bphist = midp.tile([128, T + 2, 4], f32)
            nc.vector.memset(bphist[:], 0.0)

            # ================= Stage A: projection =================
            with tc.tile_pool(name="xin", bufs=3) as xinp, \
                 tc.tile_pool(name="xT", bufs=4) as xTp, \
                 tc.tile_pool(name="pp", bufs=3) as ppp, \
                 tc.tile_pool(name="po", bufs=3) as pop, \
                 tc.tile_pool(name="pstr", bufs=4, space="PSUM") as pstrp, \
                 tc.tile_pool(name="pspot", bufs=2, space="PSUM") as pspotp, \
                 tc.tile_pool(name="ptp", bufs=2, space="PSUM") as ptpp:
                for tt in range(NTOK // 128):
                    b, ch = tt // 8, tt % 8
                    xtile = xinp.tile([128, D], f32)
                    nc.sync.dma_start(xtile[:], x_d[tt * 128:(tt + 1) * 128, :])
                    pspot = pspotp.tile([64, 128], f32)
                    for dc in range(8):
                        pstr = pstrp.tile([128, 128], f32)
                        nc.tensor.transpose(
                            pstr[:], xtile[:, dc * 128:(dc + 1) * 128], i128[:])
                        xT = xTp.tile([128, 128], f32)
                        if dc % 2 == 0:
                            nc.vector.tensor_copy(xT[:], pstr[:])
                        else:
                            nc.scalar.copy(xT[:], pstr[:])
                        nc.tensor.matmul(pspot[:], lhsT=wsb[:, dc, :], rhs=xT[:],
                                         start=(dc == 0), stop=(dc == 7))
                    # bias add PSUM->SBUF
                    pp = ppp.tile([64, 128], f32)
                    nc.vector.tensor_scalar(pp[:], pspot[:], bcol[:], None, OP.add)
                    # boundary energies (mask all ones): t=0 / t=T-1
                    if ch == 0:
                        nc.vector.tensor_scalar(pp[:, 0:1], pp[:, 0:1],
                                                lrsb[:, 0:1], None, OP.add)
                    if ch == 7:
                        nc.vector.tensor_scalar(pp[:, 127:128], pp[:, 127:128],
                                                lrsb[:, 1:2], None, OP.add)
                    # two slot copies into potT2
                    slots = [(g, l) for g in range(2) for l in range(8)
                             if SLOTB[g][l] == b]
                    for k, (g, l) in enumerate(slots):
                        dst = potT2[g * 64:(g + 1) * 64, l,
                                    ch * 128:(ch + 1) * 128]
                        if k == 0:
                            nc.vector.tensor_copy(dst, pp[:])
                        else:
                            nc.scalar.copy(dst, pp[:])
                    # pot output: transpose [64u,128t] -> [128t, 64u]
                    ptp = ptpp.tile([128, 64], f32)
                    nc.tensor.transpose(ptp[:], pp[:], i128[0:64, 0:64])
                    po = pop.tile([128, 64], f32)
                    nc.scalar.copy(po[:], ptp[:])
                    nc.sync.dma_start(
                        pot_d[b, ch * 128:(ch + 1) * 128, :], po[:])

            # ================= Stage C: forward =================
            with tc.tile_pool(name="sc", bufs=3) as scp, \
                 tc.tile_pool(name="mx", bufs=8) as mxp, \
                 tc.tile_pool(name="zz", bufs=2) as zzp, \
                 tc.tile_pool(name="pstr2", bufs=3, space="PSUM") as ps2p:

                mstate = None
                pending = []  # (ps_tr, maxfix, s) awaiting bp extraction

                def emit_bp(ps_tr, maxfix, s):
                    mfneg = mxp.tile([128, 4], f32, name="mfneg")
                    nc.vector.tensor_scalar(mfneg[:], maxfix[:], -HUGE, None,
                                            OP.mult)
                    z1 = zzp.tile([128, 4, 64], f32, name="z1")
                    v = ps_tr[:].rearrange("p (a b) -> p a b", a=4)
                    for c in range(4):
                        nc.scalar.activation(z1[:, c, :], v[:, c, :], AF.Abs,
                                             bias=mfneg[:, c:c + 1], scale=HUGE)
                    if BPMODE == "ttr":
                        for c in range(4):
                            nc.vector.tensor_tensor_reduce(
                                out=z1[:, c, :], in0=wrev[:, c, :],
                                in1=z1[:, c, :],
                                scale=1.0, scalar=-3.0e38,
                                op0=OP.subtract, op1=OP.max,
                                accum_out=bphist[:, s, c:c + 1])
                    else:
                        nc.vector.tensor_tensor(out=z1[:], in0=wrev[:],
                                                in1=z1[:], op=OP.subtract)
                        nc.vector.tensor_reduce(bphist[:, s, :], z1[:],
                                                axis=AX, op=OP.max)

                for s in range(1, FWDSTEPS + 1):
                    scores = scp.tile([128, 4, 64], f32, name="scores")
                    if s == 1:
                        in0 = potT2[:, 0:4, 0:1].broadcast_to([128, 4, 64])
                    else:
                        in0 = mstate[:].unsqueeze(2).broadcast_to([128, 4, 64])
                    in1 = transbc[:] if s < T else zero4[:]
                    nc.vector.tensor_tensor(out=scores[:], in0=in0, in1=in1,
                                            op=OP.add)
                    ps_tr = ps2p.tile([128, 256], f32, name="ps_tr")
                    sflat = scores[:].rearrange("p a b -> p (a b)")
                    nc.tensor.transpose(ps_tr[:, 0:128], sflat[:, 0:128],
                                        i128[:])
                    nc.tensor.transpose(ps_tr[:, 128:256], sflat[:, 128:256],
                                        i128[:])
                    maxfix = mxp.tile([128, 4], f32, name="maxfix")
                    nc.vector.tensor_reduce(
                        maxfix[:], ps_tr[:].rearrange("p (a b) -> p a b", a=4),
                        axis=AX, op=OP.max)
                    if s < T:
                        mstate = mxp.tile([128, 4], f32, name="mstate")
                        off = 4 * (s % 2)
                        nc.vector.tensor_tensor(
                            out=mstate[:], in0=maxfix[:],
                            in1=potT2[:, off:off + 4, s], op=OP.add)
                    pending.append((ps_tr, maxfix, s))
                    if len(pending) > 1:
                        emit_bp(*pending.pop(0))
                for args in pending:
                    emit_bp(*args)

            # ================= Stage D: bp fix + relayout =================
            dpool = tc.alloc_tile_pool(name="dpool", bufs=1, side="right")
            # bphist holds (63 - bp); fix in place
            bpv = bphist[:].rearrange("p t a -> p (t a)")
            nc.vector.tensor_scalar(bpv, bpv, -1.0, 63.0, OP.mult, OP.add)

            # bpT1 [64 r, 8 b, 2 P, 8 blk, 64 j] u8 ; s = 2*(blk*64+r) + P
            bpT1 = dpool.tile([64, 8, 2, 8, 64], u8)
            bph4 = bphist[:, 0:T, :].rearrange("p (t par) a -> p t par a",
                                               par=2)
            with tc.tile_pool(name="psd", bufs=2, space="PSUM") as psdp:
                for P in range(2):
                    for blk in range(8):
                        psd = psdp.tile([64, 512], f32)
                        for g in range(2):
                            for h in range(4):
                                bb = MS[P][g][h]
                                src = bph4[g * 64:(g + 1) * 64,
                                           blk * 64:(blk + 1) * 64, P, h]
                                dst = psd[:, bb * 64:(bb + 1) * 64]
                                if g == 0:
                                    nc.tensor.transpose(dst, src,
                                                        i64dup[0:64, :])
                                else:
                                    nc.tensor.matmul(dst, lhsT=src,
                                                     rhs=i64dup[64:128, :],
                                                     start=True, stop=True,
                                                     skip_group_check=True)
                        dst = bpT1[:, :, P, blk, :]
                        if blk % 2 == 0:
                            nc.vector.tensor_copy(
                                dst, psd[:].rearrange("r (b x) -> r b x", b=8))
                        else:
                            nc.scalar.copy(
                                dst, psd[:].rearrange("r (b x) -> r b x", b=8))

            bpnat8 = dpool.tile([8, 64, 2, 8, 64], u8)
            for b in range(8):
                nc.sync.dma_start(bpnat8[b:b + 1], bpT1[:, b, :, :, :])

            # ================= Stage E: backward chase =================
            midp.release()
            with tc.tile_pool(name="chs", bufs=1) as chp:
                taghist = chp.tile([8, 1024], f32)
                ohjunk = chp.tile([8, 64], f32)
                # final tags: bphist[:, 1024]: parity 0 -> M0: b = 4g+h, j=0 row
                for b in range(8):
                    g, h = b // 4, b % 4
                    nc.sync.dma_start(taghist[b:b + 1, 1023:1024],
                                      bphist[g * 64:g * 64 + 1, 1024, h:h + 1])
                for s in range(T - 1, 0, -1):
                    r, blk, P = (s // 2) % 64, (s // 2) // 64, s % 2
                    nc.vector.scalar_tensor_tensor(
                        out=ohjunk[:], in0=iota8[:], scalar=taghist[:, s:s + 1],
                        in1=bpnat8[:, r, P, blk, :],
                        op0=OP.is_equal, op1=OP.mult,
                        accum_out=taghist[:, s - 1:s])
                # bulk onehot: taghist [8,1024] -> tgT [128, 8c, 8b]
                tgT = chp.tile([128, 8, 8], f32)
                ohbig = chp.tile([128, 8, 8, 64], f32)
                with tc.tile_pool(name="pst", bufs=2, space="PSUM") as pstp:
                    for c in range(8):
                        pst = pstp.tile([128, 8], f32)
                        nc.tensor.transpose(
                            pst[:], taghist[:, c * 128:(c + 1) * 128],
                            i128[0:8, 0:8])
                        if c % 2 == 0:
                            nc.vector.tensor_copy(tgT[:, c, :], pst[:])
                        else:
                            nc.scalar.copy(tgT[:, c, :], pst[:])
                nc.vector.tensor_tensor(
                    out=ohbig[:],
                    in0=tgT[:].unsqueeze(3).broadcast_to([128, 8, 8, 64]),
                    in1=iotaU[:].unsqueeze(1).unsqueeze(1).broadcast_to(
                        [128, 8, 8, 64]),
                    op=OP.is_equal)
                for b in range(8):
                    nc.sync.dma_start(
                        oh_d[b].rearrange("(c r) u -> r c u", c=8),
                        ohbig[:, :, b, :])
            dpool.release()

    nc.finalize()
    return nc


def _consts():
    i128 = np.eye(128, dtype=np.float32)
    iota8 = np.tile(np.arange(64, dtype=np.float32)[None, :], (8, 1))
    iotaU = np.tile(np.arange(64, dtype=np.float32)[None, :], (128, 1))
    wrev = np.tile((63 - np.arange(64, dtype=np.float32))[None, None, :],
                   (128, 4, 1)).reshape(128, 256)
    return i128, iota8, iotaU, wrev


def kernel(inputs, mask, W, b, trans, left_b, right_b):
    from concourse.bass_utils import run_bass_kernel_spmd

    if "nc" not in _cached:
        _cached["nc"] = _build_nc()
    nc = _cached["nc"]

    inputs = np.ascontiguousarray(np.asarray(inputs, np.float32))
    W = np.ascontiguousarray(np.asarray(W, np.float32))
    bvec = np.asarray(b, np.float32).reshape(U, 1)
    trans = np.asarray(trans, np.float32)
    lr = np.stack([np.asarray(left_b, np.float32),
                   np.asarray(right_b, np.float32)], axis=1)
    i128, iota8, iotaU, wrev = _consts()
    # transbc [128, 256]: p=(g,i), cols (h,j): trans[i, j]
    i_of_p = np.arange(128) % 64
    transbc = np.tile(trans[i_of_p][:, None, :], (1, 4, 1)).reshape(128, 256)

    in_maps = []
    for c in range(8):
        shard = inputs[c * NB:(c + 1) * NB].reshape(NTOK, D)
        in_maps.append({
            "x": np.ascontiguousarray(shard),
            "w": W, "bcol": bvec,
            "lr": np.ascontiguousarray(lr),
            "i128": i128,
            "transbc": np.ascontiguousarray(transbc.astype(np.float32)),
            "wrev": np.ascontiguousarray(wrev),
            "iota8": np.ascontiguousarray(iota8),
            "iotaU": np.ascontiguousarray(iotaU),
        })

    trace = bool(int(os.environ.get("CRF_TRACE", "0")))
    res = run_bass_kernel_spmd(nc, in_maps, core_ids=list(range(8)),
                               trace=trace)
    if trace:
        print("HW exec time:", res.exec_time_ns, "ns")
        print("mean exec:", res.mean_exec_time_ns, "trace:",
              res.instructions_and_trace[1] if res.instructions_and_trace else None)
    pot = np.concatenate([r["pot_out"] for r in res.results], axis=0)
    onehot = np.concatenate([r["oh_out"] for r in res.results], axis=0)
    return pot, onehot
